# revision 88
# baseline (speedup 1.0000x reference)
"""CausalWanS2V self-attention (sparse_attention) — 8-core head-sharded Trainium2 Bass kernel.

v2: bf16 operand pipeline, exact-720 moving dims, pair-batched exp,
head-interleaved slab blocks, projections as PE fillers inside the
attention stream, denominator adds split DVE/Pool by count.

Layout strategy (per core c, heads 2c..2c+1):
  - Projections head-dim-major: qT/kT [hd=128, s=720] from lhsT = W^T tiles
    (host pre-transposed bf16), rhs = x^T tiles (host pre-transposed bf16).
  - qk RMSNorm over the full 2048-dim vector -> AllGather of per-core
    square-sum rows [1, 720] f32 (tiny), then rsqrt on-device.
  - RoPE in T-layout via pair-swap permutation matmul + DVE muls against
    host-prepared (norm-weight x cos/sin, sign-folded) bf16 tables.
  - Attention in S^T layout: for each 128-row kpos tile, S^T = kT.T @ rq
    (PE bf16, f32 PSUM), two tiles' scores packed in one 3-bank PSUM pair
    tile; E = exp(scale*S^T) over the pair in ONE ACT op -> bf16 SBUF;
    O^T += v.T @ E (PE, accumulating PSUM); denominators via DVE/Pool
    f32 adds (count-split) + final ones-matmul.
  - Heads interleave in two-slab blocks (h0 s01, h1 s01, h0 s23, ...) with
    psO evicted to SBUF between blocks, so PE filler work (k/v projections)
    spreads across the whole window and PE never idles (p-state).
  - o-projection per s-tile from OT bf16; partial [720, 2048] bf16 out;
    host sums the 8 partials + o_b in f32.
"""
import sys

sys.path.insert(0, "/opt/trn_rl_repo")

import numpy as np
import concourse.bass as bass
import concourse.mybir as mybir
import concourse.tile as tile
from concourse import bacc
from concourse import bass_utils

f32 = mybir.dt.float32
bf16 = mybir.dt.bfloat16
AF = mybir.ActivationFunctionType

SEQ = 720
DIM = 2048
NH = 16
HD = 128
CACHE = 11520
N_CORES = 8
HPC = NH // N_CORES        # heads per core = 2
HDC = HPC * HD             # 256 out dims per core
OLD = CACHE - SEQ          # 10800 old cache rows
NTILE_OLD = 85             # old rows padded to 85 tiles of 128
OLDP = NTILE_OLD * 128     # 10880
SLAB_T = 17                # kpos tiles per DMA slab
SLAB = SLAB_T * 128        # 2176
NSLAB = 5
SM_SCALE = float(HD) ** -0.5
EPS = 1e-6
KT = DIM // 128            # 16 contraction tiles

S_TILES = [(i * 128, min(128, SEQ - i * 128)) for i in range(6)]  # 5x128+80
SC0 = ((0, 512), (512, 208))    # psum-bank-legal chunks, pair sub-tile 0
SC1 = ((0, 304), (304, 416))    # pair sub-tile 1 (offset 720 in the 3-bank tile)
PAIR_SC = (SC0, SC1)
N_WARM = 16                     # PE p-state warm-up matmuls


def _emit(nc, tc, d):
    ap = {k: v.ap() for k, v in d.items()}

    with nc.allow_low_precision(reason="bf16 pipeline; rel tol 2e-2"), \
         tc.tile_pool(name="p0", bufs=1) as p0, \
         tc.tile_pool(name="dram", bufs=1, space="DRAM") as dpool, \
         tc.tile_pool(name="psS", bufs=2, space="PSUM") as psS, \
         tc.tile_pool(name="psO", bufs=1, space="PSUM") as psO, \
         tc.tile_pool(name="epool", bufs=10) as epool, \
         tc.tile_pool(name="katt", bufs=4) as katt, \
         tc.tile_pool(name="vatt", bufs=4) as vatt:

        # ---- persistent tiles ----
        rq = [p0.tile([128, SEQ], bf16, tag=f"rq{h}", name=f"rq{h}") for h in range(HPC)]
        rk = [p0.tile([128, SEQ], bf16, tag=f"rk{h}", name=f"rk{h}") for h in range(HPC)]
        qb = {}
        for tn in ("q", "k"):
            for h in range(HPC):
                qb[(tn, h)] = p0.tile([128, SEQ], bf16, tag=f"{tn}b{h}", name=f"{tn}b{h}")
        vs = [p0.tile([128, HDC], bf16, tag=f"vs{st}", name=f"vs{st}") for st in range(6)]
        OT = [p0.tile([128, SEQ], bf16, tag=f"OT{h}", name=f"OT{h}") for h in range(HPC)]
        o_sb = [p0.tile([128, SEQ], f32, tag=f"osb{h}", name=f"osb{h}") for h in range(HPC)]
        accD = [p0.tile([128, SEQ], f32, tag=f"accD{h}", name=f"accD{h}") for h in range(HPC)]
        accP = [p0.tile([128, SEQ], f32, tag=f"accP{h}", name=f"accP{h}") for h in range(HPC)]
        kb_sb = p0.tile([128, SEQ], f32, tag="kb_sb")
        ones_col = p0.tile([128, 1], f32, tag="ones_col")
        ones_col_b = p0.tile([128, 1], bf16, tag="ones_col_b")
        ones_row = p0.tile([1, 128], bf16, tag="ones_row")
        warm_rhs = p0.tile([1, 512], bf16, tag="warm_rhs")
        eps_t = p0.tile([1, 1], f32, tag="eps_t")
        prewarm = p0.tile([1, 1], f32, tag="prewarm")
        recipf = [p0.tile([1, SEQ], bf16, tag=f"recipf{i}", name=f"recipf{i}")
                  for i in range(2)]
        nc.gpsimd.memset(ones_row[:], 1.0)
        nc.gpsimd.memset(warm_rhs[:], 1.0)
        nc.gpsimd.memset(ones_col[:], 1.0)
        nc.gpsimd.memset(ones_col_b[:], 1.0)
        nc.gpsimd.memset(eps_t[:], EPS)
        # pre-load the natural_log_exp act table while DMAs stream
        nc.scalar.activation(prewarm[:], eps_t[:], AF.Exp)

        # ---- PE p-state warm-up: dependency-free bf16 matmuls ----
        warm_ps = psS.tile([128, SEQ], f32, tag="s", name="warm")
        for i in range(N_WARM):
            nc.tensor.matmul(warm_ps[:, 0:512], ones_row[:], warm_rhs[:],
                             start=True, stop=True)

        # ---- input tiles + DMA order (= schedule priority) ----
        xt = p0.tile([128, KT, SEQ], bf16, tag="xt")
        wq = p0.tile([128, KT, HDC], bf16, tag="wq")
        wk = p0.tile([128, KT, HDC], bf16, tag="wk")
        wv = p0.tile([128, KT, HDC], bf16, tag="wv")
        owt = p0.tile([128, HPC, DIM], bf16, tag="owt")
        cw = {nm: p0.tile([128, HPC * SEQ], bf16, tag=nm, name=nm)
              for nm in ("cosq", "sinq", "cosk", "sink")}
        bias_t = p0.tile([128, 4], f32, tag="bias")
        vb_t = p0.tile([1, HDC], bf16, tag="vb")
        swap_t = p0.tile([128, 128], bf16, tag="swap")
        ones_r = p0.tile([1, SEQ], bf16, tag="ones_r")
        x_r = ap["xT"].rearrange("(g p) s -> p g s", p=128)
        w_rs = {n: ap[n].rearrange("(g p) n -> p g n", p=128)
                for n in ("wqT", "wkT", "wvT")}

        nc.sync.dma_start(wq[:], w_rs["wqT"])
        nc.sync.dma_start(bias_t[:], ap["qk_bias"])
        nc.sync.dma_start(swap_t[:], ap["swap"])
        nc.sync.dma_start(ones_r[:], ap["ones_r"])
        nc.sync.dma_start(xt[:, 0:4, :], x_r[:, 0:4, :])
        nc.sync.dma_start(xt[:, 4:8, :], x_r[:, 4:8, :])
        nc.sync.dma_start(xt[:, 8:12, :], x_r[:, 8:12, :])
        nc.sync.dma_start(xt[:, 12:16, :], x_r[:, 12:16, :])
        nc.sync.dma_start(wk[:], w_rs["wkT"])
        nc.sync.dma_start(cw["cosq"][:], ap["cosq"])
        nc.sync.dma_start(cw["sinq"][:], ap["sinq"])

        # slab tiles: allocated on demand, 4-deep rotation per pool
        slab_k = {}
        slab_v = {}
        probe = p0.tile([1, 1], bf16, tag="probe")

        def prefetch_slab(h, j, gate=False):
            if (h, j) in slab_k:
                return
            ks = katt.tile([128, SLAB], bf16, tag="ks", name=f"ks{h}_{j}")
            vsl = vatt.tile([128, SLAB_T, HD], bf16, tag="vsl", name=f"vs{h}_{j}")
            if gate:
                nc.vector.tensor_copy(ks[0:1, 0:1], probe[:])
                nc.vector.tensor_copy(vsl[0:1, 0:1, 0:1], probe[:])
            nc.sync.dma_start(ks[:], ap["kTold"][h, :, j * SLAB:(j + 1) * SLAB])
            nc.sync.dma_start(vsl[:], ap["vold"][h, :, j * SLAB_T:(j + 1) * SLAB_T, :])
            slab_k[(h, j)] = ks
            slab_v[(h, j)] = vsl

        # k slab (0,0) ungated (needed by the first S matmul ~t+6us);
        # its v half and wk go behind the collective gate below
        ks00 = katt.tile([128, SLAB], bf16, tag="ks", name="ks0_0")
        nc.sync.dma_start(ks00[:], ap["kTold"][0, :, 0:SLAB])
        vs00 = vatt.tile([128, SLAB_T, HD], bf16, tag="vsl", name="vs0_0")
        slab_k[(0, 0)] = ks00
        slab_v[(0, 0)] = vs00

        # ---- helpers ----
        def proj_psum(ps_t, wt, h, groups, start, stop):
            """Accumulate x^T @ W^T[h] for the given contraction groups."""
            for gi, g in enumerate(groups):
                for off, n in SC0:
                    nc.tensor.matmul(
                        ps_t[:, off:off + n],
                        wt[:, g, h * HD:(h + 1) * HD],
                        xt[:, g, off:off + n],
                        start=(start and gi == 0), stop=(stop and gi == len(groups) - 1))

        def rowsum_sq(row_ps, sq_t, start, stop):
            for off, n in SC0:
                nc.tensor.matmul(row_ps[0:1, off:off + n], ones_col_b[:],
                                 sq_t[:, off:off + n], start=start, stop=stop)

        def launch_allgather(tn, row_ps):
            """Evict row -> DRAM bounce -> AllGather -> SBUF [8, 720].
            Bounce copies on the idle SP queue; the collective itself stays
            on the Pool queue (gates the non-critical gpsimd loads)."""
            partial_sb = p0.tile([1, SEQ], f32, tag=f"partial{tn}", name=f"partial{tn}")
            nc.scalar.copy(partial_sb[0:1, :], row_ps[0:1, :])
            bounce_in = dpool.tile([1, SEQ], f32, name=f"bin{tn}")
            bounce_out = dpool.tile([N_CORES, SEQ], f32, name=f"bout{tn}")
            nc.sync.dma_start(bounce_in[:], partial_sb[:])
            nc.gpsimd.collective_compute(
                "AllGather", mybir.AluOpType.bypass,
                replica_groups=[list(range(N_CORES))],
                ins=[bounce_in.opt()], outs=[bounce_out.opt()])
            g_t = p0.tile([N_CORES, SEQ], f32, tag=f"gth{tn}", name=f"gth{tn}")
            nc.sync.dma_start(g_t[:], bounce_out[:])
            return g_t

        def rope_head(tn, h):
            """qb[(tn,h)] <- roped(qb[(tn,h)]) (collective-independent part)."""
            sw_ps = psS.tile([128, SEQ], f32, tag="s", name=f"sw_{tn}{h}")
            for off, n in SC0:
                nc.tensor.matmul(sw_ps[:, off:off + n], swap_t[:],
                                 qb[(tn, h)][:, off:off + n], start=True, stop=True)
            cos_t = cw["cosq" if tn == "q" else "cosk"]
            sin_t = cw["sinq" if tn == "q" else "sink"]
            qbsw = p0.tile([128, SEQ], bf16, tag="qbsw")
            nc.vector.tensor_copy(qbsw[:], sw_ps[:])
            t1 = p0.tile([128, SEQ], bf16, tag="t1")
            nc.vector.tensor_mul(t1[:], qb[(tn, h)][:],
                                 cos_t[:, h * SEQ:(h + 1) * SEQ])
            nc.vector.tensor_mul(qbsw[:], qbsw[:], sin_t[:, h * SEQ:(h + 1) * SEQ])
            nc.vector.tensor_add(qb[(tn, h)][:], t1[:], qbsw[:])

        def norm_finalize_a(tn, ti, g_t):
            """Global mean-square -> rsqrt factor (ACT chain)."""
            g_b = p0.tile([N_CORES, SEQ], bf16, tag=f"gb{tn}", name=f"gb{tn}")
            nc.vector.tensor_copy(g_b[:], g_t[:])
            sums_ps = psS.tile([1, SEQ], f32, tag="s", name=f"sums{tn}")
            for off, n in SC0:
                nc.tensor.matmul(sums_ps[0:1, off:off + n], ones_col_b[0:N_CORES, :],
                                 g_b[:, off:off + n], start=True, stop=True)
            ln_t = p0.tile([1, SEQ], f32, tag=f"ln{tn}", name=f"ln{tn}")
            nc.scalar.activation(ln_t[:], sums_ps[0:1, :], AF.Ln,
                                 scale=1.0 / DIM, bias=eps_t[:])
            nc.scalar.activation(recipf[ti][:], ln_t[:], AF.Exp, scale=-0.5)

        def norm_finalize_b(tn, ti, out_tiles, use_psO=False):
            # q-path: the broadcast lives in psO (free pre-phase) so pair 0's
            # score-slot allocation doesn't wait on the rq-h1 multiply
            pool = psO if use_psO else psS
            fb_ps = pool.tile([128, SEQ], f32, tag="o" if use_psO else "s",
                              name=f"fb{tn}")
            for off, n in SC0:
                nc.tensor.matmul(fb_ps[:, off:off + n], ones_row[:],
                                 recipf[ti][0:1, off:off + n], start=True, stop=True)
            for h in range(HPC):
                nc.vector.tensor_mul(out_tiles[h][:], qb[(tn, h)][:], fb_ps[:])

        def norm_finalize(tn, ti, g_t, out_tiles):
            norm_finalize_a(tn, ti, g_t)
            norm_finalize_b(tn, ti, out_tiles)

        # ---- filler work units (consumed between attention pairs) ----
        fillers = []
        kp_state = {"row": None}

        def mk_kproj(h, g0):
            def f():
                ps = psS.tile([128, SEQ], f32, tag="s", name=f"kp{h}_{g0}")
                proj_psum(ps, wk, h, list(range(g0, g0 + 4)), start=True, stop=True)
                if g0 == 0:
                    nc.vector.tensor_copy(kb_sb[:], ps[:])
                elif g0 < 12:
                    nc.vector.tensor_add(kb_sb[:], kb_sb[:], ps[:])
                else:
                    nc.vector.tensor_add(kb_sb[:], kb_sb[:], ps[:])
                    nc.scalar.activation(sq_t[h][:], kb_sb[:], AF.Square,
                                         bias=bias_t[:, 2 + h:3 + h], scale=1.0)
                    nc.vector.tensor_scalar_add(qb[("k", h)][:], kb_sb[:],
                                                bias_t[:, 2 + h:3 + h])
            return f

        def mk_krow():
            def f():
                row_k = psS.tile([1, SEQ], f32, tag="s", name="row_k")
                for h in range(HPC):
                    rowsum_sq(row_k, sq_t[h], start=(h == 0), stop=(h == HPC - 1))
                kp_state["row"] = launch_allgather("k", row_k)
            return f

        def mk_vproj(st):
            s0, m = S_TILES[st]
            def f():
                vp = psS.tile([128, HDC], f32, tag="s", name=f"vp{st}")
                for g in range(KT):
                    nc.tensor.matmul(vp[0:m, :], xt[:, g, s0:s0 + m], wv[:, g, :],
                                     start=(g == 0), stop=False)
                nc.tensor.matmul(vp[0:m, :], ones_r[0:1, s0:s0 + m], vb_t[:],
                                 start=False, stop=True)
                nc.vector.tensor_copy(vs[st][0:m, :], vp[0:m, :])
            return f

        def mk_ropek(h):
            return lambda: rope_head("k", h)

        def mk_knorm_a():
            return lambda: norm_finalize_a("k", 1, kp_state["row"])

        def mk_knorm_b():
            return lambda: norm_finalize_b("k", 1, rk)

        # k-proj h0 first (h0's new-token tiles come first), then h1
        # NOTE: kb_sb is shared; a head's 4 fillers must be consecutive.
        for h in range(HPC):
            for g0 in range(0, KT, 4):
                fillers.append(mk_kproj(h, g0))
        fillers.append(mk_krow())
        for st in range(3):
            fillers.append(mk_vproj(st))
        for h in range(HPC):
            fillers.append(mk_ropek(h))
        fillers.append(mk_knorm_a())
        fillers.append(mk_knorm_b())
        # 16 fillers over blocks A-D (68 pairs): one every 4 pairs;
        # the last three v-proj s-tiles fill block E's otherwise-idle PE
        e_fillers = [mk_vproj(st) for st in range(3, 6)]
        f_fillers = []

        # ---- pre-phase: q projection -> AllGather(q) -> rq ----
        qp = [psS.tile([128, SEQ], f32, tag="s", name="qp0"),
              psO.tile([128, SEQ], f32, tag="o", name="qp1")]
        for g in range(KT):
            for h in range(HPC):
                proj_psum(qp[h], wq, h, [g], start=(g == 0), stop=(g == KT - 1))
        sq_t = [p0.tile([128, SEQ], bf16, tag=f"sq{h}", name=f"sq{h}") for h in range(HPC)]
        for h in range(HPC):
            # square on ACT (reads the psum directly) so the DVE bias-cast
            # is off the row-sum critical path; q/k bias is zero here so the
            # square of the pre-bias value is exact enough regardless
            nc.scalar.activation(sq_t[h][:], qp[h][:], AF.Square,
                                 bias=bias_t[:, h:h + 1], scale=1.0)
            nc.vector.tensor_scalar_add(qb[("q", h)][:], qp[h][:],
                                        bias_t[:, h:h + 1])
        row_q = psS.tile([1, SEQ], f32, tag="s", name="row_q")
        for h in range(HPC):
            rowsum_sq(row_q, sq_t[h], start=(h == 0), stop=(h == HPC - 1))
        gth_q = launch_allgather("q", row_q)
        for h in range(HPC):
            rope_head("q", h)
        # h0's k-projection fills the collective wait with real PE work
        for fi in range(4):
            fillers[fi]()
        norm_finalize_a("q", 0, gth_q)
        # h1's k-projection overlaps the Ln/Exp chain
        for fi in range(4, 8):
            fillers[fi]()
        filler_i_start = 8
        norm_finalize_b("q", 0, rq, use_psO=True)
        # non-critical loads, gated behind the q-collective with an actual
        # WAW dependency (1-element probe write fed from g_t) — the tile
        # scheduler hoists wait-free DMA triggers, so queue order alone
        # does not keep these off the DMA engines during the bounce
        nc.vector.tensor_copy(probe[:], gth_q[0:1, 0:1])
        nc.vector.tensor_copy(slab_v[(0, 0)][0:1, 0:1, 0:1], probe[:])
        nc.sync.dma_start(slab_v[(0, 0)][:], ap["vold"][0, :, 0:SLAB_T, :])
        # wv early: the first v-proj filler otherwise stalls PE on it
        nc.vector.tensor_copy(wv[0:1, 0:1, 0:1], probe[:])
        nc.sync.dma_start(wv[:], w_rs["wvT"])
        prefetch_slab(0, 1, gate=True)
        for t, src in ((cw["cosk"], ap["cosk"]), (cw["sink"], ap["sink"])):
            nc.vector.tensor_copy(t[0:1, 0:1], probe[:])
            nc.sync.dma_start(t[:], src)
        nc.vector.tensor_copy(vb_t[0:1, 0:1], probe[:])
        nc.sync.dma_start(vb_t[:], ap["v_bias"])
        nc.vector.tensor_copy(owt[0:1, 0:1, 0:1], probe[:])
        nc.sync.dma_start(owt[:], ap["owT"].rearrange("(h p) n -> p h n", p=128))

        # ---- attention: head-interleaved two-slab blocks ----
        # per-head accumulation chains for the softmax denominators
        first_flag = {}
        for h in range(HPC):
            first_flag[("D", h)] = True
            first_flag[("P", h)] = True
        add_ctr = {"n": 0}

        def acc_op(h, e_t, ti, m):
            i = add_ctr["n"]
            add_ctr["n"] += 1
            # ~104 DVE / 78 Pool split (4 of 7 -> DVE); last few to DVE
            # so the Pool queue drains before the final denominator
            if (i % 7 < 4) if i < 164 else (i % 3 < 2):
                eng, a_t, key = nc.vector, accD[h], ("D", h)
            else:
                eng, a_t, key = nc.gpsimd, accP[h], ("P", h)
            if first_flag[key]:
                first_flag[key] = False
                eng.tensor_copy(a_t[0:m, :], e_t[0:m, ti, :])
            else:
                eng.tensor_add(a_t[0:m, :], a_t[0:m, :], e_t[0:m, ti, :])

        pend = []          # queued (bid, e_t, ti, lhsT_v, m) PV/acc work
        blkstate = {}      # per-block psO bookkeeping (bid -> ps/first_pv/h)

        def drain(k):
            while len(pend) > k:
                bid, e_t, ti, lv, m = pend.pop(0)
                st = blkstate[bid]
                last = (st["left"] == 1)
                st["left"] -= 1
                for off, n in SC0:
                    nc.tensor.matmul(st["ps"][:, off:off + n], lv,
                                     e_t[0:m, ti, off:off + n],
                                     start=st["first_pv"], stop=last)
                st["first_pv"] = False
                acc_op(st["h"], e_t, ti, m)

        def drain_block(bid):
            while pend and pend[0][0] == bid:
                drain(len(pend) - 1)

        def do_pair(bid, h, tiles, split_exp=False):
            """tiles: list of 1-2 (lhsT_k, lhsT_v, m)."""
            s_ps = psS.tile([128, 2, SEQ], f32, tag="s")
            for ti, (lk, lv, m) in enumerate(tiles):
                for off, n in PAIR_SC[ti]:
                    nc.tensor.matmul(s_ps[0:m, ti, off:off + n], lk,
                                     rq[h][:, off:off + n], start=True, stop=True)
            e_t = epool.tile([128, 2, SEQ], bf16, tag="e")
            nt = len(tiles)
            if split_exp and nt == 2:
                # ramp pairs: per-sub-tile exps so the first PV unblocks early
                for ti in range(nt):
                    nc.scalar.activation(e_t[:, ti:ti + 1, :],
                                         s_ps[:, ti:ti + 1, :],
                                         AF.Exp, scale=SM_SCALE)
            else:
                nc.scalar.activation(e_t[:, 0:nt, :], s_ps[:, 0:nt, :],
                                     AF.Exp, scale=SM_SCALE)
            for ti, (lk, lv, m) in enumerate(tiles):
                pend.append((bid, e_t, ti, lv, m))
            drain(12)

        def slab_tiles(h, j):
            ks, vsl = slab_k[(h, j)], slab_v[(h, j)]
            out = []
            for t in range(SLAB_T):
                m = 48 if (j == 4 and t == 16) else 128
                out.append((ks[:, t * 128:t * 128 + m], vsl[0:m, t, :], m))
            return out

        def new_tiles(h):
            return [(rk[h][:, s0:s0 + m], vs[st][0:m, h * HD:(h + 1) * HD], m)
                    for st, (s0, m) in enumerate(S_TILES)]

        evict_count = {0: 0, 1: 0}

        def close_block(bid):
            """Drain the block's PV backlog, evict psO into o_sb."""
            drain_block(bid)
            h = blkstate[bid]["h"]
            o_ps = blkstate[bid]["ps"]
            if evict_count[h] == 0:
                nc.vector.tensor_copy(o_sb[h][:], o_ps[:])
            else:
                nc.vector.tensor_add(o_sb[h][:], o_sb[h][:], o_ps[:])
            evict_count[h] += 1

        def denom_stage1(h):
            d_ps = psS.tile([1, SEQ], f32, tag="s", name=f"d{h}")
            for off, n in SC0:
                nc.tensor.matmul(d_ps[0:1, off:off + n], ones_col[:],
                                 accD[h][:, off:off + n], start=True, stop=False)
                nc.tensor.matmul(d_ps[0:1, off:off + n], ones_col[:],
                                 accP[h][:, off:off + n], start=False, stop=True)
            rec_d = p0.tile([1, SEQ], bf16, tag=f"rec{h}", name=f"rec{h}")
            nc.vector.reciprocal(rec_d[:], d_ps[0:1, :])
            return rec_d

        def denom_stage2(h, rec_d):
            fb2 = psS.tile([128, SEQ], f32, tag="s", name=f"fb2_{h}")
            for off, n in SC0:
                nc.tensor.matmul(fb2[:, off:off + n], ones_row[:],
                                 rec_d[0:1, off:off + n], start=True, stop=True)
            nc.vector.tensor_mul(OT[h][:], o_sb[h][:], fb2[:])

        filler_i = {"n": filler_i_start, "pairs": 0}
        post_q = []        # priority work consumed between pairs (block close,
                           # denominator stages of the previous block)

        def tick_filler():
            filler_i["pairs"] += 1
            if post_q:
                post_q.pop(0)()
                return
            if filler_i["pairs"] % 4 == 0 and filler_i["n"] < len(fillers):
                fillers[filler_i["n"]]()
                filler_i["n"] += 1

        # block schedule: (head, slabs, include_new)
        blocks = [
            (0, (0, 1), False),
            (1, (0, 1), False),
            (0, (2, 3), False),
            (1, (2, 3), False),
            (0, (4,), True),
            (1, (4,), True),
        ]
        denom_pend = {}
        for bi, (h, slabs, with_new) in enumerate(blocks):
            nxt = bi + 1
            if nxt < len(blocks):
                nh, nslabs, _ = blocks[nxt]
                for j in nslabs:
                    prefetch_slab(nh, j)
            tiles = []
            for j in slabs:
                tiles.extend(slab_tiles(h, j))
            if with_new:
                tiles.extend(new_tiles(h))
            npairs = (len(tiles) + 1) // 2
            o_ps = psO.tile([128, SEQ], f32, tag="o")
            blkstate[bi] = {"ps": o_ps, "first_pv": True, "h": h,
                            "left": len(tiles)}
            # schedule the PREVIOUS block's close + its denom work between
            # this block's first pairs so PE never sits behind the chain
            if bi > 0:
                ph = blocks[bi - 1][0]
                post_q.append(lambda b=bi - 1: close_block(b))
                if blocks[bi - 1][2]:
                    post_q.append(lambda hh=ph: denom_pend.__setitem__(
                        hh, denom_stage1(hh)))
                    post_q.append(lambda hh=ph: denom_stage2(hh, denom_pend[hh]))
            if bi == 4:
                post_q.extend(e_fillers)
            if bi == 5:
                post_q.extend(f_fillers)
            for i in range(0, len(tiles), 2):
                do_pair(bi, h, tiles[i:i + 2])
                tick_filler()
        # final block close + denominators
        close_block(len(blocks) - 1)
        rec1 = denom_stage1(1)
        denom_stage2(1, rec1)

        # ---- o-projection (per-core partial, bf16 out) ----
        # three 2-chunk psum units rotate (2x psS slots + the freed psO
        # slot) so matmuls, copies and DMAs pipeline deeper; h-major order
        # lets h0 partials pre-run during the h1 denominator chain
        uctr = 0
        for sti, (s0, m) in enumerate(S_TILES):
            out_sb = p0.tile([128, DIM], bf16, tag=f"out_sb{sti % 2}",
                             name=f"out_sb{sti}")
            for half in range(2):
                if uctr % 3 == 2:
                    op_ps = psO.tile([128, 2, 512], f32, tag="o")
                else:
                    op_ps = psS.tile([128, 2, 512], f32, tag="s")
                uctr += 1
                for h in range(HPC):
                    for k in range(2):
                        ci = half * 2 + k
                        nc.tensor.matmul(op_ps[0:m, k, :], OT[h][:, s0:s0 + m],
                                         owt[:, h, ci * 512:(ci + 1) * 512],
                                         start=(h == 0), stop=(h == HPC - 1))
                # one 2-chunk copy per half, alternating DVE/ACT
                # (gpsimd cannot access PSUM on hardware)
                if half == 0:
                    nc.vector.tensor_copy(out_sb[0:m, 0:1024], op_ps[0:m, :, :])
                else:
                    nc.scalar.copy(out_sb[0:m, 1024:2048], op_ps[0:m, :, :])
                # fire the half's output dma as soon as its copies land
                nc.sync.dma_start(
                    ap["out"][s0:s0 + m, half * 1024:(half + 1) * 1024],
                    out_sb[0:m, half * 1024:(half + 1) * 1024])


def _patch_act_tables(nc):
    """All ACT funcs used here (Exp, Ln, Copy) live in act-func-set 6
    (natural_log_exp_and_others); retarget every load to set 6 and drop
    redundant ones (saves ~1.3us per avoided switch)."""
    for blk in nc.main_func.blocks:
        keep = []
        seen = False
        for ins in blk.instructions:
            if isinstance(ins, mybir.InstLoadActFuncSet):
                ins.act_func_set_id = 6
                si = ins.sync_info
                clean = si is None or (len(si.on_wait) == 0 and len(si.on_update) == 0)
                if seen and clean:
                    continue
                seen = True
            keep.append(ins)
        blk.instructions[:] = keep


def _build():
    nc = bacc.Bacc("TRN2", target_bir_lowering=False, debug=False,
                   num_devices=N_CORES)
    d = {}
    d["xT"] = nc.dram_tensor("xT", [DIM, SEQ], bf16, kind="ExternalInput")
    d["wqT"] = nc.dram_tensor("wqT", [DIM, HDC], bf16, kind="ExternalInput")
    d["wkT"] = nc.dram_tensor("wkT", [DIM, HDC], bf16, kind="ExternalInput")
    d["wvT"] = nc.dram_tensor("wvT", [DIM, HDC], bf16, kind="ExternalInput")
    d["owT"] = nc.dram_tensor("owT", [HDC, DIM], bf16, kind="ExternalInput")
    d["qk_bias"] = nc.dram_tensor("qk_bias", [128, 4], f32, kind="ExternalInput")
    d["v_bias"] = nc.dram_tensor("v_bias", [1, HDC], bf16, kind="ExternalInput")
    d["swap"] = nc.dram_tensor("swap", [128, 128], bf16, kind="ExternalInput")
    d["ones_r"] = nc.dram_tensor("ones_r", [1, SEQ], bf16, kind="ExternalInput")
    d["cosq"] = nc.dram_tensor("cosq", [128, HPC * SEQ], bf16, kind="ExternalInput")
    d["sinq"] = nc.dram_tensor("sinq", [128, HPC * SEQ], bf16, kind="ExternalInput")
    d["cosk"] = nc.dram_tensor("cosk", [128, HPC * SEQ], bf16, kind="ExternalInput")
    d["sink"] = nc.dram_tensor("sink", [128, HPC * SEQ], bf16, kind="ExternalInput")
    d["kTold"] = nc.dram_tensor("kTold", [HPC, 128, OLDP], bf16, kind="ExternalInput")
    d["vold"] = nc.dram_tensor("vold", [HPC, 128, NTILE_OLD, HD], bf16,
                               kind="ExternalInput")
    d["out"] = nc.dram_tensor("out", [SEQ, DIM], bf16, kind="ExternalOutput")
    with tile.TileContext(nc) as tc:
        _emit(nc, tc, d)
    nc.compile()
    _patch_act_tables(nc)
    return nc


_NC_CACHE = None


def _get_nc():
    global _NC_CACHE
    if _NC_CACHE is None:
        _NC_CACHE = _build()
    return _NC_CACHE


def _to_bf16(a):
    import ml_dtypes
    return np.asarray(a, dtype=np.float32).astype(ml_dtypes.bfloat16)


def _prep_inputs(x, q_w, q_b, k_w, k_b, v_w, v_b, o_w, o_b, norm_q_w, norm_k_w,
                 cache_k, cache_v, freqs_cos, freqs_sin,
                 current_start, frame_seqlen, sink_tokens):
    cs, sink = int(current_start), int(sink_tokens)
    rolling = CACHE - sink
    local_start = (cs - sink) % rolling + sink
    old_idx = np.r_[0:local_start, local_start + SEQ:CACHE]
    assert old_idx.size == OLD

    xT = np.ascontiguousarray(np.asarray(x)[0].T)          # [2048, 720]

    # RoPE/norm tables in T layout: cos_full[d, s] = cos[s, d//2] * w[d];
    # sin_full[d, s] = sin[s, d//2] * w[d^1] * (-1 if d even else +1)
    dd = np.arange(HD)
    fc = np.asarray(freqs_cos)
    fs = np.asarray(freqs_sin)
    cos_d = fc.T[dd // 2, :]                               # [128, 720]
    sin_d = fs.T[dd // 2, :]
    sign = np.where(dd % 2 == 0, -1.0, 1.0).astype(np.float32)[:, None]
    swap_m = np.zeros((HD, HD), dtype=np.float32)
    swap_m[dd, dd ^ 1] = 1.0

    ck = np.asarray(cache_k[0])                            # [11520, 16, 128]
    cv = np.asarray(cache_v[0])
    ck_old = np.zeros((OLDP, NH, HD), dtype=np.float32)
    cv_old = np.zeros((OLDP, NH, HD), dtype=np.float32)
    ck_old[:OLD] = ck[old_idx]
    cv_old[:OLD] = cv[old_idx]

    in_maps = []
    for c in range(N_CORES):
        hs = slice(c * HDC, (c + 1) * HDC)
        heads = [c * HPC + h for h in range(HPC)]
        bias4 = np.zeros((128, 4), dtype=np.float32)
        for h in range(HPC):
            bias4[:, 0 + h] = np.asarray(q_b)[hs][h * HD:(h + 1) * HD]
            bias4[:, 2 + h] = np.asarray(k_b)[hs][h * HD:(h + 1) * HD]
        cosq = np.empty((128, HPC * SEQ), dtype=np.float32)
        sinq = np.empty((128, HPC * SEQ), dtype=np.float32)
        cosk = np.empty((128, HPC * SEQ), dtype=np.float32)
        sink_t = np.empty((128, HPC * SEQ), dtype=np.float32)
        for h in range(HPC):
            wqn = np.asarray(norm_q_w)[hs][h * HD:(h + 1) * HD]
            wkn = np.asarray(norm_k_w)[hs][h * HD:(h + 1) * HD]
            sl = slice(h * SEQ, (h + 1) * SEQ)
            cosq[:, sl] = cos_d * wqn[:, None]
            sinq[:, sl] = sin_d * wqn[dd ^ 1][:, None] * sign
            cosk[:, sl] = cos_d * wkn[:, None]
            sink_t[:, sl] = sin_d * wkn[dd ^ 1][:, None] * sign
        kT_old = np.ascontiguousarray(
            ck_old[:, heads, :].transpose(1, 2, 0))        # [2, 128, 10880]
        # v in [p, t, e] layout: v[t*128+p, e] -> [2, 128, 85, 128]
        v_old = np.ascontiguousarray(
            cv_old[:, heads, :].reshape(NTILE_OLD, 128, HPC, HD)
            .transpose(2, 1, 0, 3))
        in_maps.append({
            "xT": _to_bf16(xT),
            "wqT": _to_bf16(np.asarray(q_w)[hs, :].T),
            "wkT": _to_bf16(np.asarray(k_w)[hs, :].T),
            "wvT": _to_bf16(np.asarray(v_w)[hs, :].T),
            "owT": _to_bf16(np.asarray(o_w)[:, hs].T),
            "qk_bias": bias4,
            "v_bias": _to_bf16(np.asarray(v_b)[hs].reshape(1, HDC)),
            "swap": _to_bf16(swap_m),
            "ones_r": _to_bf16(np.ones((1, SEQ), dtype=np.float32)),
            "cosq": _to_bf16(cosq), "sinq": _to_bf16(sinq),
            "cosk": _to_bf16(cosk), "sink": _to_bf16(sink_t),
            "kTold": _to_bf16(kT_old),
            "vold": _to_bf16(v_old),
        })
    return in_maps


def run_spmd(in_maps, **kw):
    nc = _get_nc()
    return bass_utils.run_bass_kernel_spmd(
        nc, in_maps, core_ids=list(range(N_CORES)), **kw)


def kernel(**inputs):
    inputs = {k: np.asarray(v) if not np.isscalar(v) else v
              for k, v in inputs.items()}
    in_maps = _prep_inputs(**inputs)
    res = run_spmd(in_maps)
    out = np.zeros((SEQ, DIM), dtype=np.float32)
    for c in range(N_CORES):
        out += np.asarray(res.results[c]["out"], dtype=np.float32)
    out += np.asarray(inputs["o_b"], dtype=np.float32)[None, :]
    return out[None].astype(np.float32)
